# revision 1
# baseline (speedup 1.0000x reference)
"""MultiHeadAttention Trainium2 kernel (8 NeuronCores).

Sharding: core c -> (batch b = c//2, head-group g = c%2) of the 12 heads.
Each core computes attention for its 6 heads of one batch element and a
partial projection; the host sums the two head-group partials per batch
element and adds proj bias.

Per-core dataflow (feat-major / transposed layouts to avoid transposes):
  x [S,768] --PE-transpose--> xT [768,S]
  qT/kT = (wqk^T @ xT) + bias     (float32r matmuls, TF32-class)
  v [S,384] seq-major (+ ones column for softmax denominators)
  scoresT[sk,sq] = kT_chunk^T-pair @ qT  (2 heads packed in PE row groups)
  pT = exp(scoresT/8)              (ScalarE, no max subtraction - bounded)
  avT[65,sq] = [v|1]^T @ pT        (row 64 = softmax denominator)
  attn_outT = avT[0:64] * (1/avT[64]) broadcast via K=1 PE matmul
  yT[768,S] = wp^T @ attn_outT     (partial projection, host sums pairs)
"""
import sys

sys.path.insert(0, "/opt/trn_rl_repo")

import numpy as np

import concourse.bass as bass
import concourse.mybir as mybir
import concourse.tile as tile
from concourse import bacc
from concourse.bass_utils import run_bass_kernel_spmd
from concourse.masks import make_identity

F32 = mybir.dt.float32
F32R = mybir.dt.float32r
EXP = mybir.ActivationFunctionType.Exp
ADD = mybir.AluOpType.add

HID = 768
D = 64  # head dim
LHEADS = 6  # heads per core
PAIRS = 3


def build_nc(S: int, taps: bool = False):
    nc = bacc.Bacc("TRN2", target_bir_lowering=False, debug=False)
    NSEQ = S // 128  # seq chunks of 128
    NBLK = S // 512  # seq blocks of 512
    XG = 4  # x DMA chunk groups
    NXG = NSEQ // XG

    x = nc.dram_tensor("x", [S, HID], F32, kind="ExternalInput")
    wqk = nc.dram_tensor("wqk", [HID, 768], F32, kind="ExternalInput")
    wv = nc.dram_tensor("wv", [HID, 384], F32, kind="ExternalInput")
    bqk = nc.dram_tensor("bqk", [768], F32, kind="ExternalInput")
    bv = nc.dram_tensor("bv", [384], F32, kind="ExternalInput")
    wp = nc.dram_tensor("wp", [384, HID], F32, kind="ExternalInput")
    yT = nc.dram_tensor("yT", [HID, S], F32, kind="ExternalOutput")
    if taps:
        t_xT = nc.dram_tensor("t_xT", [128, 6 * S], F32, kind="ExternalOutput")
        t_qT = nc.dram_tensor("t_qT", [128, S], F32, kind="ExternalOutput")
        t_kT = nc.dram_tensor("t_kT", [128, S], F32, kind="ExternalOutput")
        t_v = nc.dram_tensor("t_v", [128, S // 128 * 2 * 65], F32, kind="ExternalOutput")
        t_pt = nc.dram_tensor("t_pt", [128, 1024], F32, kind="ExternalOutput")
        t_av = nc.dram_tensor("t_av", [65, 512], F32, kind="ExternalOutput")
        t_ao = nc.dram_tensor("t_ao", [128, 3 * S], F32, kind="ExternalOutput")
        t_rc = nc.dram_tensor("t_rc", [1, 512], F32, kind="ExternalOutput")
        t_bc = nc.dram_tensor("t_bc", [D, 512], F32, kind="ExternalOutput")

    with tile.TileContext(nc) as tc:
        with (
            tc.tile_pool(name="const", bufs=1) as cp,
            tc.tile_pool(name="wts", bufs=1) as wpool,
            tc.tile_pool(name="ao", bufs=1) as aop,
            tc.tile_pool(name="ps", bufs=2, space="PSUM") as ps,
        ):
            ident = cp.tile([128, 128], F32, tag="ident")
            make_identity(nc, ident[:])
            ones_f = cp.tile([33, 128], F32, tag="onesf")
            nc.vector.memset(ones_f[:], 1.0)
            ones_r = cp.tile([33, 128], F32R, tag="ones")
            nc.vector.tensor_copy(ones_r[:], ones_f[:])
            bqk_sb = cp.tile([128, 6], F32, tag="bqk")
            nc.sync.dma_start(bqk_sb[:], bqk[:].rearrange("(c p) -> p c", p=128))
            bv_sb = cp.tile([1, 384], F32, tag="bvs")
            nc.sync.dma_start(bv_sb[:], bv[:].rearrange("(o f) -> o f", o=1))
            bv_r = cp.tile([1, 384], F32R, tag="bvr")
            nc.vector.tensor_copy(bv_r[:], bv_sb[:])
            # load the exp ACT table off the critical path
            warm = cp.tile([1, 16], F32, tag="warm")
            nc.scalar.activation(warm[:], ones_f[0:1, 0:16], EXP, bias=0.0, scale=0.0)

            aoT = aop.tile([128, PAIRS, S], F32R, tag="aoT")

            with tc.tile_pool(name="xT", bufs=1) as xtp:
                xT = xtp.tile([128, 6, S], F32R, tag="xT")

                # --- x DMA (group 0 first), weights, transposes, qkT(0), v interleaved ---
                with tc.tile_pool(name="wstage", bufs=1) as wst, \
                     tc.tile_pool(name="xin", bufs=1) as xin:
                    x_ap = x[:].rearrange("(n p) d -> p n d", p=128)
                    x_ts = []
                    for g in range(XG):
                        x_t = xin.tile([128, NXG, HID], F32, tag=f"x{g}", name=f"x_t{g}")
                        x_ts.append(x_t)
                    nc.sync.dma_start(x_ts[0][:], x_ap[:, 0:NXG, :])
                    wqk_f = wst.tile([128, 6, 768], F32, tag="wqkf")
                    wqk_ap = wqk[:].rearrange("(c p) f -> p c f", p=128)
                    for kc in range(6):
                        nc.sync.dma_start(
                            wqk_f[:, kc : kc + 1, :], wqk_ap[:, kc : kc + 1, :]
                        )
                    for g in range(1, XG):
                        nc.sync.dma_start(
                            x_ts[g][:], x_ap[:, g * NXG : (g + 1) * NXG, :]
                        )
                    wv_f = wst.tile([128, 6, 384], F32, tag="wvf")
                    nc.sync.dma_start(
                        wv_f[:], wv[:].rearrange("(c p) f -> p c f", p=128)
                    )

                    wqk_r = wpool.tile([128, 6, 768], F32R, tag="wqkr")
                    for kc in range(6):
                        nc.vector.tensor_copy(
                            wqk_r[:, kc, :], wqk_f[:, kc, :]
                        )
                    wv_r = wpool.tile([128, 6, 384], F32R, tag="wvr")
                    nc.vector.tensor_copy(wv_r[:], wv_f[:])


                    # transposes: per x-group, per hid-chunk j, 4 seq chunks into
                    # one psum tile, then a single [128, 512] contiguous copy
                    for g in range(XG):
                        for j in range(6):
                            tp = ps.tile([128, NXG, 128], F32, tag="av", bufs=2)
                            for i in range(NXG):
                                nc.tensor.transpose(
                                    tp[:, i, :],
                                    x_ts[g][:, i, j * 128 : (j + 1) * 128],
                                    ident[:],
                                )
                            dst = xT[:, j, g * NXG * 128 : (g + 1) * NXG * 128]
                            if (g * 6 + j) % 2 == 0:
                                nc.scalar.copy(dst, tp[:])
                            else:
                                nc.vector.tensor_copy(dst, tp[:])

                if taps:
                    nc.sync.dma_start(t_xT[:], xT[:].bitcast(F32).rearrange("p a b -> p (a b)"))
                den_init = [0]
                with (
                    tc.tile_pool(name="qk", bufs=2) as qkp,
                    tc.tile_pool(name="vv", bufs=2) as vvp,
                    tc.tile_pool(name="pt", bufs=4) as ptp,
                    tc.tile_pool(name="sm", bufs=2) as smp,
                ):
                    vsl = None
                    for pj in range(PAIRS):
                        # ---- qT/kT for this pair: [128 feats, S] f32r ----
                        qTp = qkp.tile([128, S], F32R, tag="qT")
                        kTp = qkp.tile([128, S], F32R, tag="kT")
                        for n in range(NBLK):
                            for dst, wcol in ((kTp, 3 + pj), (qTp, pj)):
                                qp = ps.tile([128, 512], F32, tag="qk", bufs=1)
                                for k in range(6):
                                    nc.tensor.matmul(
                                        qp[:],
                                        wqk_r[:, k, wcol * 128 : (wcol + 1) * 128],
                                        xT[:, k, n * 512 : (n + 1) * 512],
                                        start=(k == 0),
                                        stop=(k == 5),
                                    )
                                nc.vector.tensor_scalar(
                                    dst[:, n * 512 : (n + 1) * 512],
                                    qp[:],
                                    bqk_sb[:, wcol : wcol + 1],
                                    None,
                                    ADD,
                                )

                        if pj == 1:
                            wp_r = wpool.tile([128, 3, HID], F32R, tag="wpr")
                            wp_ap = wp[:].rearrange("(c p) f -> p c f", p=128)
                            for kc in range(3):
                                wp_f = smp.tile(
                                    [128, 1, HID], F32, tag="wpf", bufs=1,
                                    name=f"wp_f{kc}",
                                )
                                nc.sync.dma_start(wp_f[:], wp_ap[:, kc : kc + 1, :])
                                nc.vector.tensor_copy(wp_r[:, kc : kc + 1, :], wp_f[:])

                        if pj == 0:
                            # ---- v for all 6 heads (emitted after pair-0 qkT) ----
                            vsl = vvp.tile([128, NSEQ, 6, D + 1], F32R, tag="v", bufs=1)
                            vones = smp.tile([128, NSEQ, 6, 1], F32, tag="vones")
                            nc.vector.memset(vones[:], 1.0)
                            nc.vector.tensor_copy(vsl[:, :, :, D : D + 1], vones[:])
                            for i in range(NSEQ):
                                vp = ps.tile([128, 512], F32, tag="qk", bufs=1)
                                for k in range(6):
                                    nc.tensor.matmul(
                                        vp[:, 0:384],
                                        xT[:, k, i * 128 : (i + 1) * 128],
                                        wv_r[:, k, :],
                                        start=(k == 0),
                                        stop=False,
                                    )
                                nc.tensor.matmul(
                                    vp[:, 0:384],
                                    ones_r[0:1, :],
                                    bv_r[0:1, :],
                                    start=False,
                                    stop=True,
                                )
                                nc.vector.tensor_copy(
                                    vsl[:, i, :, 0:D],
                                    vp[:, 0:384].rearrange("p (h d) -> p h d", h=6),
                                )

                        if taps and pj == 0:
                            nc.sync.dma_start(t_qT[:], qTp[:].bitcast(F32))
                            nc.sync.dma_start(t_kT[:], kTp[:].bitcast(F32))
                            nc.sync.dma_start(t_v[:], vsl[:].bitcast(F32).rearrange("p a b c -> p (a b c)"))
                        # ---- attention for the two heads of this pair ----
                        for n in range(NBLK):
                            avs = [
                                ps.tile([D + 1, 512], F32, tag="av", bufs=2, name=f"av{hi}")
                                for hi in range(2)
                            ]
                            for sk in range(NSEQ):
                                sc = ps.tile([128, 2, 512], F32, tag="sc")
                                for hi in range(2):
                                    nc.tensor.matmul(
                                        sc[:, hi, :],
                                        kTp[
                                            hi * D : (hi + 1) * D,
                                            sk * 128 : (sk + 1) * 128,
                                        ],
                                        qTp[
                                            hi * D : (hi + 1) * D,
                                            n * 512 : (n + 1) * 512,
                                        ],
                                        start=True,
                                        stop=True,
                                    )
                                pt = ptp.tile([128, 2, 512], F32R, tag="pt")
                                nc.scalar.activation(
                                    pt[:], sc[:], EXP, bias=0.0, scale=0.125
                                )
                                if taps and pj == 0 and n == 0 and sk == 0:
                                    nc.sync.dma_start(t_pt[:], pt[:].bitcast(F32).rearrange("p a b -> p (a b)"))
                                for hi in range(2):
                                    nc.tensor.matmul(
                                        avs[hi][:],
                                        vsl[:, sk, 2 * pj + hi, :],
                                        pt[:, hi, :],
                                        start=(sk == 0),
                                        stop=(sk == NSEQ - 1),
                                    )
                            # drain av psum to SBUF right away (frees the psum
                            # slot for the next block) then normalize from SBUF
                            av_sbs = []
                            for hi in range(2):
                                av_sb = smp.tile([D + 1, 512], F32, tag="avsb", name=f"av_sb{hi}")
                                nc.vector.tensor_copy(av_sb[:], avs[hi][:])
                                av_sbs.append(av_sb)
                            if taps and pj == 0 and n == 0:
                                nc.sync.dma_start(t_av[:], av_sbs[0][:])
                            den = smp.tile([33, 512], F32, tag="den")
                            if den_init[0] < 2:
                                den_init[0] += 1
                                nc.vector.memset(den[:], 1.0)
                            for hi in range(2):
                                nc.vector.tensor_copy(
                                    den[32 * hi : 32 * hi + 1, :],
                                    av_sbs[hi][D : D + 1, :],
                                )
                            rec_sb = smp.tile([33, 512], F32, tag="rec")
                            nc.vector.reciprocal(rec_sb[:], den[:])
                            rec_r = smp.tile([33, 512], F32R, tag="recr")
                            nc.vector.tensor_copy(rec_r[:], rec_sb[:])
                            for hi in range(2):
                                bc = ps.tile([D, 512], F32, tag="bc", bufs=1)
                                nc.tensor.matmul(
                                    bc[:],
                                    ones_r[32 * hi : 32 * hi + 1, 0:D],
                                    rec_r[32 * hi : 32 * hi + 1, :],
                                    start=True,
                                    stop=True,
                                )
                                if taps and pj == 0 and n == 0 and hi == 0:
                                    nc.sync.dma_start(t_rc[:], rec_r[0:1, :].bitcast(F32))
                                nc.vector.tensor_mul(
                                    aoT[
                                        hi * D : (hi + 1) * D,
                                        pj,
                                        n * 512 : (n + 1) * 512,
                                    ],
                                    av_sbs[hi][0:D, :],
                                    bc[:],
                                )

            if taps:
                nc.sync.dma_start(t_ao[:], aoT[:].bitcast(F32).rearrange("p a b -> p (a b)"))
            # ---- projection: yT[768, S] = wp^T @ aoT (partial) ----
            yT_ap = yT[:].rearrange("(c p) s -> p c s", p=128)
            with tc.tile_pool(name="yt", bufs=6) as ytp:
                for n in range(NBLK):
                    for m in range(6):
                        pp = ps.tile([128, 512], F32, tag="qk", bufs=1)
                        for k in range(3):
                            nc.tensor.matmul(
                                pp[:],
                                wp_r[:, k, m * 128 : (m + 1) * 128],
                                aoT[:, k, n * 512 : (n + 1) * 512],
                                start=(k == 0),
                                stop=(k == 2),
                            )
                        yt_t = ytp.tile([128, 512], F32, tag="yT")
                        if m % 2 == 0:
                            nc.scalar.copy(yt_t[:], pp[:])
                        else:
                            nc.vector.tensor_copy(yt_t[:], pp[:])
                        nc.sync.dma_start(
                            yT_ap[:, m, n * 512 : (n + 1) * 512], yt_t[:]
                        )

    nc.finalize()
    return nc


_NC_CACHE = {}


def _get_nc(S, taps=False):
    key = (S, taps)
    if key not in _NC_CACHE:
        _NC_CACHE[key] = build_nc(S, taps)
    return _NC_CACHE[key]


def kernel(x, qkv_w, qkv_b, proj_w, proj_b, return_res=False, **run_kwargs):
    x = np.asarray(x, dtype=np.float32)
    qkv_w = np.asarray(qkv_w, dtype=np.float32)
    qkv_b = np.asarray(qkv_b, dtype=np.float32)
    proj_w = np.asarray(proj_w, dtype=np.float32)
    proj_b = np.asarray(proj_b, dtype=np.float32)
    B, S, _ = x.shape

    nc = _get_nc(S)
    in_maps = []
    for c in range(8):
        b, g = c // 2, c % 2
        qs = slice(384 * g, 384 * g + 384)
        ks = slice(768 + 384 * g, 768 + 384 * g + 384)
        vs = slice(1536 + 384 * g, 1536 + 384 * g + 384)
        in_maps.append(
            {
                "x": np.ascontiguousarray(x[b]),
                "wqk": np.ascontiguousarray(
                    np.concatenate([qkv_w[:, qs], qkv_w[:, ks]], axis=1)
                ),
                "wv": np.ascontiguousarray(qkv_w[:, vs]),
                "bqk": np.ascontiguousarray(
                    np.concatenate([qkv_b[qs], qkv_b[ks]])
                ),
                "bv": np.ascontiguousarray(qkv_b[vs]),
                "wp": np.ascontiguousarray(proj_w[384 * g : 384 * g + 384, :]),
            }
        )
    try:
        res = run_bass_kernel_spmd(
            nc, in_maps, core_ids=list(range(8)), **run_kwargs
        )
    except Exception:
        # transient NRT/device errors happen occasionally; retry once
        res = run_bass_kernel_spmd(
            nc, in_maps, core_ids=list(range(8)), **run_kwargs
        )
    out = np.empty((B, S, HID), np.float32)
    for b in range(B):
        yt = res.results[2 * b]["yT"] + res.results[2 * b + 1]["yT"]
        out[b] = yt.T + proj_b
    if return_res:
        return out, res
    return out



# revision 18
# speedup vs baseline: 1.2814x; 1.2814x over previous
"""MultiHeadAttention Trainium2 kernel (8 NeuronCores).

Sharding: core c -> (batch b = c//2, head-group g = c%2) of the 12 heads.
Each core computes attention for its 6 heads of one batch element and a
partial projection; the host sums the two head-group partials per batch
element and adds proj bias.

v2 design (fp8 DoubleRow attention, bf16 projections):
  host ships xT [768,S] bf16 (pre-transposed), wqk/wv pre-scaled x16 bf16,
  wp pre-scaled 1/64 bf16 -> no PE transposes, no on-device weight converts.
  qT/kT = fp8(16*(w^T x + b)) via bf16 matmuls + DVE drain     [feat-major]
  v     = fp8(16*(x^T w + b)) seq-major, 65th col = 0.25 (denominator row)
  scores: fp8 DoubleRow matmuls, second sub-row zeroed (2x PE rate)
  pt    = fp8(exp(scores * 0.125/256)) on ACT (the critical engine)
  av    : fp8 DoubleRow contracting two 128-seq chunks per matmul (4x rate)
  attn  = av[0:64] * (1/av[64]) via DVE reciprocal + K=1 broadcast matmul
  yT    = wp^T @ attn in bf16 (partial; host sums the two head-group parts)

PE work is interleaved into the attention loop via an ordered filler queue
so the ACT engine (exp) stays saturated; it is the roofline.
"""
import sys

sys.path.insert(0, "/opt/trn_rl_repo")

import ml_dtypes
import numpy as np

import concourse.bass as bass
import concourse.mybir as mybir
import concourse.tile as tile
from concourse import bacc
from concourse.bass_utils import run_bass_kernel_spmd

F32 = mybir.dt.float32
F32R = mybir.dt.float32r
BF16 = mybir.dt.bfloat16
F8 = mybir.dt.float8e4
DR = mybir.MatmulPerfMode.DoubleRow
EXP = mybir.ActivationFunctionType.Exp
ADD = mybir.AluOpType.add

BF_NP = ml_dtypes.bfloat16

HID = 768
D = 64  # head dim
LHEADS = 6  # heads per core
PAIRS = 3
EXPSCALE = 0.125 / 256.0  # q,k carry x16 each


def build_nc(S: int):
    nc = bacc.Bacc("TRN2", target_bir_lowering=False, debug=False)
    NSEQ = S // 128
    NBLK = S // 512
    NJ = NSEQ // 2  # sk chunk-pairs per block

    xTd = nc.dram_tensor("xT", [HID, S], BF16, kind="ExternalInput")
    wqkd = nc.dram_tensor("wqk", [HID, 768], BF16, kind="ExternalInput")
    wvd = nc.dram_tensor("wv", [HID, 384], BF16, kind="ExternalInput")
    bqkd = nc.dram_tensor("bqk", [768], F32, kind="ExternalInput")
    bvd = nc.dram_tensor("bv", [384], F32, kind="ExternalInput")
    wpd = nc.dram_tensor("wp", [384, HID], BF16, kind="ExternalInput")
    yTd = nc.dram_tensor("yT", [HID, S], BF16, kind="ExternalOutput")

    with tile.TileContext(nc) as tc:
        with (
            tc.tile_pool(name="const", bufs=1) as cp,
            tc.tile_pool(name="wts", bufs=1) as wpool,
            tc.tile_pool(name="xsb", bufs=1) as xtp,
            tc.tile_pool(name="qk8", bufs=2) as qkp,
            tc.tile_pool(name="vv", bufs=1) as vvp,
            tc.tile_pool(name="pt", bufs=11) as ptp,
            tc.tile_pool(name="sm", bufs=3) as smp,
            tc.tile_pool(name="ao", bufs=1) as aop,
            tc.tile_pool(name="yt", bufs=4) as ytp,
            tc.tile_pool(name="psq", bufs=2, space="PSUM") as psq,
            tc.tile_pool(name="pssc", bufs=2, space="PSUM") as pssc,
            tc.tile_pool(name="psav", bufs=2, space="PSUM") as psav,
        ):
            # ---- constants ----
            ones_f = cp.tile([1, 16], F32, tag="onesf")
            nc.vector.memset(ones_f[:], 1.0)
            # exp ACT table load off the critical path
            warm = cp.tile([1, 16], F32, tag="warm")
            nc.scalar.activation(warm[:], ones_f[:], EXP, bias=0.0, scale=0.0)
            bqk_sb = cp.tile([128, 6], F32, tag="bqk")

            # ---- weights / x DMA, ordered so pj0's k-proj starts earliest ----
            wqk_sb = wpool.tile([128, 6, 768], BF16, tag="wqk")
            wqk_ap = wqkd[:].rearrange("(c p) f -> p c f", p=128)
            xT = xtp.tile([128, 6, S], BF16, tag="xT")
            xT_ap = xTd[:].rearrange("(c p) s -> p c s", p=128)
            wv_sb = wpool.tile([128, 6, 384], BF16, tag="wv")
            bv_sb = cp.tile([1, 384], F32, tag="bvs")

            def dma_wcol(w):
                nc.sync.dma_start(
                    wqk_sb[:, :, w * 128 : (w + 1) * 128],
                    wqk_ap[:, :, w * 128 : (w + 1) * 128],
                )

            def dma_xg(g):
                nc.sync.dma_start(
                    xT[:, :, g * 512 : (g + 1) * 512],
                    xT_ap[:, :, g * 512 : (g + 1) * 512],
                )

            dma_wcol(3)  # k cols of pair 0
            dma_xg(0)
            dma_wcol(0)  # q cols of pair 0
            nc.sync.dma_start(bqk_sb[:], bqkd[:].rearrange("(c p) -> p c", p=128))
            nc.sync.dma_start(wv_sb[:], wvd[:].rearrange("(c p) f -> p c f", p=128))
            nc.sync.dma_start(bv_sb[:], bvd[:].rearrange("(o f) -> o f", o=1))
            # ramp the PE out of its cold p-state while DMAs land: ~3us of
            # dependency-free junk matmuls so the real startup matmuls run
            # at full clock.
            junk = cp.tile([1, 512], F32, tag="junk")
            nc.vector.memset(junk[:], 0.0)
            pewarm = psq.tile([128, 512], F32, tag="qk")
            for _ in range(3):
                nc.tensor.matmul(
                    pewarm[0:16, :], ones_f[:], junk[:], start=True, stop=True
                )
            dma_xg(1)
            dma_wcol(4)
            dma_wcol(1)
            dma_xg(2)
            dma_wcol(5)
            dma_wcol(2)
            dma_xg(3)
            # broadcast v bias across partitions once (Pool is idle)
            bv_bc = cp.tile([128, 384], F32, tag="bvbc")
            nc.gpsimd.partition_broadcast(bv_bc[:], bv_sb[:], channels=128)

            # v (seq-major) for all 6 heads; col D holds 0.25 (denominator
            # row), col D+1 is zero pad. Row stride 72 keeps the dual-fp8
            # weight-load sub-row stride (6*72=432B) 16-byte aligned.
            VROW = D + 8
            vsl = vvp.tile([128, NSEQ, 6, VROW], F8, tag="v")
            nc.gpsimd.memset(vsl[:, :, :, D : D + 1], 0.25)
            nc.gpsimd.memset(vsl[:, :, :, D + 1 : D + 2], 0.0)

            aoT = aop.tile([128, PAIRS, S], BF16, tag="aoT")
            wp_sb = wpool.tile([128, 3, HID], BF16, tag="wp")

            # ---------------- emission helpers ----------------
            def qkproj_block(dst8, wcol, n):
                """dst8[:, 0, n*512:+512] = fp8(16*(w^T x + b)) for col chunk."""
                qp = psq.tile([128, 512], F32, tag="qk")
                for c in range(6):
                    nc.tensor.matmul(
                        qp[:],
                        wqk_sb[:, c, wcol * 128 : (wcol + 1) * 128],
                        xT[:, c, n * 512 : (n + 1) * 512],
                        start=(c == 0),
                        stop=(c == 5),
                    )
                nc.vector.tensor_scalar(
                    dst8[:, 0, n * 512 : (n + 1) * 512],
                    qp[:],
                    bqk_sb[:, wcol : wcol + 1],
                    None,
                    ADD,
                )

            def vproj_chunk(i):
                vp = psq.tile([128, 512], F32, tag="qk")
                for c in range(6):
                    nc.tensor.matmul(
                        vp[:, 0:384],
                        xT[:, c, i * 128 : (i + 1) * 128],
                        wv_sb[:, c, :],
                        start=(c == 0),
                        stop=(c == 5),
                    )
                nc.vector.scalar_tensor_tensor(
                    vsl[:, i, :, 0:D],
                    vp[:, 0:384].rearrange("p (h d) -> p h d", h=6),
                    1.0,
                    bv_bc[:].rearrange("p (h d) -> p h d", h=6),
                    mybir.AluOpType.mult,
                    ADD,
                )

            def wp_dma():
                nc.sync.dma_start(
                    wp_sb[:], wpd[:].rearrange("(c p) f -> p c f", p=128)
                )

            yT_ap = yTd[:].rearrange("(c p) s -> p c s", p=128)

            def proj_block(n, m):
                pp = psq.tile([128, 512], F32, tag="qk")
                for c in range(3):
                    nc.tensor.matmul(
                        pp[:],
                        wp_sb[:, c, m * 128 : (m + 1) * 128],
                        aoT[:, c, n * 512 : (n + 1) * 512],
                        start=(c == 0),
                        stop=(c == 2),
                    )
                yt = ytp.tile([128, 512], BF16, tag="yt")
                nc.vector.tensor_copy(yt[:], pp[:])
                nc.sync.dma_start(yT_ap[:, m, n * 512 : (n + 1) * 512], yt[:])

            # ---- ordered filler queue with prereq forcing ----
            queue: list[tuple[tuple, object]] = []
            emitted: set[tuple] = set()

            def force(key):
                while key not in emitted and queue:
                    k, fn = queue.pop(0)
                    fn()
                    emitted.add(k)

            def pop_one():
                if queue:
                    k, fn = queue.pop(0)
                    fn()
                    emitted.add(k)

            def flush():
                while queue:
                    pop_one()

            # ---- startup: first pair's k/q for block 0 emitted inline ----
            qT8s, kT8s = {}, {}

            def alloc_qk(pj):
                qT8 = qkp.tile([128, 2, S], F8, tag="q8", name=f"q8_{pj}")
                kT8 = qkp.tile([128, 2, S], F8, tag="k8", name=f"k8_{pj}")
                if pj < 2:  # zero DR sub-row 1 of each physical buffer once
                    nc.gpsimd.memset(qT8[:, 1, :], 0.0)
                    nc.gpsimd.memset(kT8[:, 1, :], 0.0)
                qT8s[pj], kT8s[pj] = qT8, kT8
                return qT8, kT8

            qT8, kT8 = alloc_qk(0)
            qkproj_block(kT8, 3 + 0, 0)
            qkproj_block(qT8, 0, 0)
            for n in range(1, NBLK):
                queue.append((("k", 0, n), lambda n=n: qkproj_block(kT8s[0], 3, n)))
            for i in range(NSEQ):
                queue.append((("v", i), lambda i=i: vproj_chunk(i)))
            for n in range(1, NBLK):
                queue.append((("q", 0, n), lambda n=n: qkproj_block(qT8s[0], 0, n)))
            emitted.add(("k", 0, 0))
            emitted.add(("q", 0, 0))

            # ---------------- attention: flat software-pipelined stream ----
            # step k: sc+exp for step k, av for step k-AVLAG, normalize for
            # blocks whose av finished NORMLAG steps ago. Keeps the PE queue
            # stocked with sc matmuls ahead of anything that waits on ACT/DVE
            # so the exp stream (the roofline) never stalls.
            AVLAG = 8
            NORMLAG = 2
            blocks = [
                (pj, n, hi)
                for pj in range(PAIRS)
                for n in range(NBLK)
                for hi in range(2)
            ]
            steps = [(b, J) for b in range(len(blocks)) for J in range(NJ)]
            av_tiles: dict = {}
            rec_tiles: dict = {}
            pt_hist: dict = {}
            pending_norm: list = []

            def emit_sc(b, J):
                pj, n, hi = blocks[b]
                qT8, kT8 = qT8s[pj], kT8s[pj]
                if J == 0 and hi == 0:
                    force(("q", pj, n))
                force(("k", pj, (2 * J + 1) // 4))
                sc = pssc.tile([128, 2, 512], F32, tag="sc")
                for c2 in range(2):
                    sk = 2 * J + c2
                    nc.tensor.matmul(
                        sc[:, c2, :],
                        kT8[64 * hi : 64 * hi + 64, :, sk * 128 : (sk + 1) * 128],
                        qT8[64 * hi : 64 * hi + 64, :, n * 512 : (n + 1) * 512],
                        start=True,
                        stop=True,
                        perf_mode=DR,
                    )
                pt = ptp.tile([128, 2, 512], F8, tag="pt")
                nc.scalar.activation(pt[:], sc[:], EXP, bias=0.0, scale=EXPSCALE)
                pt_hist[(b, J)] = pt

            def emit_av(b, J):
                pj, n, hi = blocks[b]
                head = 2 * pj + hi
                if J == 0:
                    av_tiles[b] = psav.tile(
                        [D + 2, 512], F32, tag="av", name=f"av{b % 2}"
                    )
                force(("v", 2 * J + 1))
                nc.tensor.matmul(
                    av_tiles[b][:],
                    vsl[:, 2 * J : 2 * J + 2, head, 0 : D + 2],
                    pt_hist.pop((b, J))[:],
                    start=(J == 0),
                    stop=(J == NJ - 1),
                    perf_mode=DR,
                )
                if J == NJ - 1:
                    rec = smp.tile([1, 512], F32, tag="rec")
                    nc.vector.reciprocal(rec[:], av_tiles[b][D : D + 1, :])
                    bcst = smp.tile([D, 512], F32, tag="bcst")
                    nc.gpsimd.partition_broadcast(bcst[:], rec[:], channels=D)
                    rec_tiles[b] = bcst

            def emit_norm(b):
                pj, n, hi = blocks[b]
                av = av_tiles.pop(b)
                bcst = rec_tiles.pop(b)
                nc.vector.tensor_mul(
                    aoT[64 * hi : 64 * hi + 64, pj, n * 512 : (n + 1) * 512],
                    av[0:D, :],
                    bcst[:],
                )
                if pj == PAIRS - 1 and hi == 1:
                    force(("wp",))
                    for m in range(6):
                        proj_block(n, m)

            NSTEP = len(steps)
            av_idx = 0
            for k in range(NSTEP + 2):
                if k < NSTEP:
                    b, J = steps[k]
                    if J == 0 and b % (2 * NBLK) == NBLK:
                        pj = b // (2 * NBLK)
                        if pj < PAIRS - 1:
                            npj = pj + 1
                            nqT8, nkT8 = alloc_qk(npj)
                            for n2 in range(NBLK):
                                queue.append(
                                    (
                                        ("k", npj, n2),
                                        lambda n2=n2, t=nkT8, w=3 + npj: qkproj_block(
                                            t, w, n2
                                        ),
                                    )
                                )
                            for n2 in range(NBLK):
                                queue.append(
                                    (
                                        ("q", npj, n2),
                                        lambda n2=n2, t=nqT8, w=npj: qkproj_block(
                                            t, w, n2
                                        ),
                                    )
                                )
                        if pj == 1:
                            queue.append((("wp",), wp_dma))
                    emit_sc(b, J)
                # av stream: lag AVLAG steps in steady state, catch up to a
                # lag of 2 over the final steps so the tail chain overlaps
                # the last exps.
                target = k - AVLAG if k < NSTEP - 6 else k - 2
                while av_idx <= min(target, NSTEP - 1):
                    b, J = steps[av_idx]
                    emit_av(b, J)
                    if J == NJ - 1:
                        pending_norm.append((k + NORMLAG, b))
                    av_idx += 1
                while pending_norm and pending_norm[0][0] <= k:
                    emit_norm(pending_norm.pop(0)[1])
                pop_one()
            while pending_norm:
                emit_norm(pending_norm.pop(0)[1])
            flush()

    nc.finalize()
    return nc


_NC_CACHE = {}


def _get_nc(S):
    if S not in _NC_CACHE:
        _NC_CACHE[S] = build_nc(S)
    return _NC_CACHE[S]


def kernel(x, qkv_w, qkv_b, proj_w, proj_b, return_res=False, **run_kwargs):
    x = np.asarray(x, dtype=np.float32)
    qkv_w = np.asarray(qkv_w, dtype=np.float32)
    qkv_b = np.asarray(qkv_b, dtype=np.float32)
    proj_w = np.asarray(proj_w, dtype=np.float32)
    proj_b = np.asarray(proj_b, dtype=np.float32)
    B, S, _ = x.shape

    nc = _get_nc(S)
    in_maps = []
    for c in range(8):
        b, g = c // 2, c % 2
        qs = slice(384 * g, 384 * g + 384)
        ks = slice(768 + 384 * g, 768 + 384 * g + 384)
        vs = slice(1536 + 384 * g, 1536 + 384 * g + 384)
        in_maps.append(
            {
                "xT": np.ascontiguousarray(x[b].T).astype(BF_NP),
                "wqk": np.ascontiguousarray(
                    16.0 * np.concatenate([qkv_w[:, qs], qkv_w[:, ks]], axis=1)
                ).astype(BF_NP),
                "wv": np.ascontiguousarray(16.0 * qkv_w[:, vs]).astype(BF_NP),
                "bqk": np.ascontiguousarray(
                    16.0 * np.concatenate([qkv_b[qs], qkv_b[ks]])
                ),
                "bv": np.ascontiguousarray(16.0 * qkv_b[vs]),
                "wp": np.ascontiguousarray(
                    proj_w[384 * g : 384 * g + 384, :] / 64.0
                ).astype(BF_NP),
            }
        )
    try:
        res = run_bass_kernel_spmd(
            nc, in_maps, core_ids=list(range(8)), **run_kwargs
        )
    except Exception:
        # transient NRT/device errors happen occasionally; retry once
        res = run_bass_kernel_spmd(
            nc, in_maps, core_ids=list(range(8)), **run_kwargs
        )
    out = np.empty((B, S, HID), np.float32)
    for b in range(B):
        yt = res.results[2 * b]["yT"].astype(np.float32) + res.results[
            2 * b + 1
        ]["yT"].astype(np.float32)
        out[b] = yt.T + proj_b
    if return_res:
        return out, res
    return out
